# revision 2
# baseline (speedup 1.0000x reference)
"""MoE audio projector kernel for 8 Trainium2 NeuronCores — sparse top-2.

The reference computes all 4 experts densely and masks with top-2 gates.
Only the top-2 experts contribute, so the device only computes
  shared-MLP(token) + gate_a * expert_a-MLP(token) + gate_b * expert_b-MLP(token)
per token.  Host does routing (rmsnorm + router softmax + top-2, exactly
the reference formulas) and sorts tokens by expert; the device is a pure
bf16 matmul pipeline.

Sharding: 8 cores x 3 segments of 768 token-slots:
  seg0: 750 of the 6000 stacked tokens, shared expert (gate 1)
  seg1+seg2: ~1500 tokens routed to expert e = core//2 (each expert's
  ~3000 tokens split across 2 cores), gate = renormalized top-2 prob.
Per segment: mm1 [5120->2048] + gelu + mm2 [2048->2048], output scaled by
the per-token gate.  Host gathers the 3 partial outputs per token and sums.

All matmuls bf16 with fp32 PSUM accumulation (1 cycle/row on the PE).
"""
import sys

sys.path.insert(0, "/opt/trn_rl_repo")

import numpy as np
import ml_dtypes

import concourse.bass as bass
import concourse.mybir as mybir
import concourse.tile as tile
from concourse.bass_utils import run_bass_kernel_spmd

BF16 = np.dtype(ml_dtypes.bfloat16)

P = 128
IN_DIM = 5120
K1 = IN_DIM // P          # 40 contraction tiles for mm1
HID = 2048
NHT = HID // P            # 16 h tiles (mm1 out) == mm2 contraction tiles
OUT_DIM = 2048
OSL = 512                 # mm2 output slice (one PSUM bank)
NOS = OUT_DIM // OSL      # 4
E = 4
SEGS = 3
SEG_TILES = 6
SEG_TOK = SEG_TILES * P   # 768
TILES = SEGS * SEG_TILES  # 18
SLOTS = SEGS * SEG_TOK    # 2304 token slots per core
TC = 384                  # mm1 token chunk (2 per segment)
NTC = SEG_TOK // TC       # 2

N_TOK = 6000              # 16 batches x 375 stacked frames
SH_PER_CORE = N_TOK // 8  # 750 shared tokens per core
EXP_CAP_CORE = 2 * SEG_TOK  # 1536 expert tokens per core
EPS_NORM = 1e-6
EPS_GATE = 1e-6

MM_DT = mybir.dt.bfloat16
F32 = mybir.dt.float32


def split_excess_waits(nc, maxw=1):
    """This container's walrus build only accepts one sync-wait command on
    CTRL-class instructions (Drain) and two on regular ones; fan extra waits
    out onto preceding same-engine NoOps."""
    for f in nc.m.functions:
        for b in f.blocks:
            newlist = []
            for inst in b.instructions:
                lim = maxw
                si = inst.sync_info
                if si is not None and si.on_wait and len(si.on_wait) > lim:
                    waits = list(si.on_wait)
                    chunks = [waits[i:i + lim] for i in range(0, len(waits), lim)]
                    for ci, ch in enumerate(chunks[:-1]):
                        d = mybir.InstNoOp(
                            name=f"{inst.name}-waitsplit{ci}",
                            ins=[], outs=[],
                            sync_info=mybir.SyncInfo(on_wait=ch, on_update=[]),
                        )
                        d.engine = inst.engine
                        nc.register_instruction(d)
                        newlist.append(d)
                    si.on_wait = chunks[-1]
                newlist.append(inst)
            b.instructions = newlist


def build_nc():
    nc = bass.Bass()
    xt = nc.dram_tensor("xt", [SEGS, P, K1, SEG_TOK], MM_DT, kind="ExternalInput")
    w1a = nc.dram_tensor("w1a", [NHT, P, K1, P], MM_DT, kind="ExternalInput")
    w1b = nc.dram_tensor("w1b", [NHT, P, K1, P], MM_DT, kind="ExternalInput")
    w2a = nc.dram_tensor("w2a", [NOS, P, NHT, OSL], MM_DT, kind="ExternalInput")
    w2b = nc.dram_tensor("w2b", [NOS, P, NHT, OSL], MM_DT, kind="ExternalInput")
    b1a = nc.dram_tensor("b1a", [P, NHT], F32, kind="ExternalInput")
    b1b = nc.dram_tensor("b1b", [P, NHT], F32, kind="ExternalInput")
    gates = nc.dram_tensor("gates", [P, TILES], F32, kind="ExternalInput")
    y = nc.dram_tensor("y", [TILES, P, OUT_DIM], MM_DT, kind="ExternalOutput")

    seg_w1 = [w1a, w1b, w1b]
    seg_w2 = [w2a, w2b, w2b]
    seg_b1 = [b1a, b1b, b1b]

    with tile.TileContext(nc) as tc:
        with (
            tc.tile_pool(name="singles", bufs=1) as singles,
            tc.tile_pool(name="xq", bufs=1) as xq_pool,
            tc.tile_pool(name="ht", bufs=1) as ht_pool,
            tc.tile_pool(name="w1s", bufs=2) as w1_pool,
            tc.tile_pool(name="w2s", bufs=2) as w2_pool,
            tc.tile_pool(name="b1s", bufs=2) as b1_pool,
            tc.tile_pool(name="outp", bufs=2) as out_pool,
            tc.tile_pool(name="psum1", bufs=2, space="PSUM") as psum1_pool,
            tc.tile_pool(name="psum2", bufs=3, space="PSUM") as psum2_pool,
        ):
            gates_sb = singles.tile([P, TILES], F32)
            nc.sync.dma_start(gates_sb, gates[:, :])

            for seg in range(SEGS):
                xT = xq_pool.tile([P, K1, SEG_TOK], MM_DT, tag="xt")
                nc.sync.dma_start(xT, xt[seg])
                b1sb = b1_pool.tile([P, NHT], F32, tag="b1")
                nc.sync.dma_start(b1sb, seg_b1[seg][:, :])

                # ---- mm1 + gelu: hT[h, tok] for the whole segment ----
                hT = ht_pool.tile([P, NHT, SEG_TOK], MM_DT, tag="ht")
                for ht in range(NHT):
                    w1blk = w1_pool.tile([P, K1, P], MM_DT, tag="w1")
                    nc.sync.dma_start(w1blk, seg_w1[seg][ht])
                    for tcb in range(NTC):
                        ts = tcb * TC
                        ps1 = psum1_pool.tile([P, TC], F32, tag="ps1")
                        for ko in range(K1):
                            nc.tensor.matmul(
                                ps1,
                                lhsT=w1blk[:, ko, :],
                                rhs=xT[:, ko, ts:ts + TC],
                                start=(ko == 0), stop=(ko == K1 - 1),
                            )
                        nc.scalar.activation(
                            out=hT[:, ht, ts:ts + TC], in_=ps1,
                            func=mybir.ActivationFunctionType.Gelu,
                            bias=b1sb[:, ht:ht + 1], scale=1.0,
                        )

                # ---- mm2 + gate scale ----
                out_sb = out_pool.tile([P, SEG_TILES, OUT_DIM], MM_DT, tag="out")
                for os_ in range(NOS):
                    w2blk = w2_pool.tile([P, NHT, OSL], MM_DT, tag="w2")
                    nc.sync.dma_start(w2blk, seg_w2[seg][os_])
                    for tt in range(SEG_TILES):
                        ps2 = psum2_pool.tile([P, OSL], F32, tag="ps2")
                        for hk in range(NHT):
                            nc.tensor.matmul(
                                ps2,
                                lhsT=hT[:, hk, tt * P:(tt + 1) * P],
                                rhs=w2blk[:, hk, :],
                                start=(hk == 0), stop=(hk == NHT - 1),
                            )
                        col = seg * SEG_TILES + tt
                        nc.vector.tensor_scalar_mul(
                            out_sb[:, tt, os_ * OSL:(os_ + 1) * OSL],
                            ps2, gates_sb[:, col:col + 1],
                        )
                for tt in range(SEG_TILES):
                    nc.sync.dma_start(y[seg * SEG_TILES + tt], out_sb[:, tt, :])

    split_excess_waits(nc)
    return nc


_NC_CACHE = {}


def _get_nc():
    if "nc" not in _NC_CACHE:
        _NC_CACHE["nc"] = build_nc()
    return _NC_CACHE["nc"]


def _routing(x, norm_w, router_w):
    """Reference-exact rmsnorm + router softmax + top-2 (jax CPU, f32).

    Returns flat normalized tokens [N_TOK, IN_DIM] f32, top-2 indices
    [N_TOK, 2] and renormalized top-2 weights [N_TOK, 2].
    """
    try:
        import jax
        import jax.numpy as jnp

        cpu = jax.devices("cpu")[0]
        with jax.default_device(cpu):
            xj = jnp.asarray(x)
            B, S, D = xj.shape
            out_len = (S - 4) // 4 + 1
            xs = xj[:, :out_len * 4, :].reshape(B, out_len, D * 4)
            var = jnp.mean(xs * xs, axis=-1, keepdims=True)
            xs = jnp.asarray(norm_w) * (xs * jax.lax.rsqrt(var + EPS_NORM))
            flat = xs.reshape(-1, D * 4)
            logits = flat @ jnp.asarray(router_w).T
            probs = jax.nn.softmax(logits.astype(jnp.float32), axis=-1)
            tw, ti = jax.lax.top_k(probs, 2)
            tw = tw / (jnp.sum(tw, axis=-1, keepdims=True) + EPS_GATE)
            return (np.asarray(flat, np.float32), np.asarray(ti),
                    np.asarray(tw, np.float32))
    except Exception:
        pass
    # numpy fallback (same formulas, f32)
    xf = np.asarray(x, np.float32)
    B, S, D = xf.shape
    out_len = (S - 4) // 4 + 1
    xs = xf[:, :out_len * 4, :].reshape(B, out_len, D * 4)
    var = np.mean(xs * xs, axis=-1, keepdims=True, dtype=np.float32)
    xs = norm_w * (xs / np.sqrt(var + EPS_NORM))
    flat = np.ascontiguousarray(xs.reshape(-1, D * 4), dtype=np.float32)
    logits = flat @ np.asarray(router_w, np.float32).T
    lmax = logits.max(axis=-1, keepdims=True)
    ex = np.exp(logits - lmax)
    probs = ex / ex.sum(axis=-1, keepdims=True)
    ti = np.argsort(-probs, kind="stable", axis=-1)[:, :2]
    tw = np.take_along_axis(probs, ti, axis=-1)
    tw = tw / (tw.sum(axis=-1, keepdims=True) + EPS_GATE)
    return flat, ti.astype(np.int32), tw.astype(np.float32)


def _prepare(x, norm_w, router_w, w1, b1, w2, b2, sw1, sb1, sw2, sb2):
    """Host routing + packing.  Returns (in_maps, meta) where meta has the
    slot->token scatter info for _unpack."""
    flat, ti, tw = _routing(
        np.asarray(x, np.float32),
        np.asarray(norm_w, np.float32),
        np.asarray(router_w, np.float32),
    )

    w1_all = np.concatenate(
        [np.asarray(w1, np.float32), np.asarray(sw1, np.float32)[None]], axis=0)
    b1_all = np.concatenate(
        [np.asarray(b1, np.float32), np.asarray(sb1, np.float32)[None]], axis=0)
    w2_all = np.concatenate(
        [np.asarray(w2, np.float32), np.asarray(sw2, np.float32)[None]], axis=0)

    # ---- slot assignment ----
    # core c: seg0 = shared tokens [c*750, (c+1)*750); segs 1-2 = expert
    # c//2's tokens, first half of that expert's list for even c, second
    # half for odd c.
    tok_idx = np.full((8, SLOTS), N_TOK, np.int64)      # pad -> zero row
    gate_val = np.zeros((8, SLOTS), np.float32)

    sh = np.arange(N_TOK)
    shared_slot = (sh // SH_PER_CORE) * SLOTS + (sh % SH_PER_CORE)
    tok_idx.reshape(-1)[shared_slot] = sh
    gate_val.reshape(-1)[shared_slot] = 1.0

    app_slot = [np.empty(N_TOK, np.int64), np.empty(N_TOK, np.int64)]
    for e in range(E):
        m0 = ti[:, 0] == e
        m1 = ti[:, 1] == e
        L = np.flatnonzero(m0 | m1)
        if len(L) > 2 * EXP_CAP_CORE:
            raise ValueError(
                f"expert {e} has {len(L)} tokens > capacity {2 * EXP_CAP_CORE}")
        g = np.where(m0[L], tw[L, 0], tw[L, 1]).astype(np.float32)
        pos = np.arange(len(L))
        core = 2 * e + (pos >= EXP_CAP_CORE)
        within = pos - (pos >= EXP_CAP_CORE) * EXP_CAP_CORE
        slots = core * SLOTS + SEG_TOK + within
        tok_idx.reshape(-1)[slots] = L
        gate_val.reshape(-1)[slots] = g
        for j in (0, 1):
            sel = np.flatnonzero(ti[:, j] == e)
            ranks = np.searchsorted(L, sel)
            app_slot[j][sel] = slots[ranks]

    # ---- device arrays ----
    flat16 = np.zeros((N_TOK + 1, IN_DIM), BF16)
    flat16[:N_TOK] = flat.astype(BF16)

    # xt per core: [SEGS, P(kp), K1(ko), SEG_TOK]
    xts = []
    for c in range(8):
        xg = flat16[tok_idx[c]]                              # [SLOTS, IN_DIM]
        xts.append(np.ascontiguousarray(
            xg.reshape(SEGS, SEG_TOK, K1, P).transpose(0, 3, 2, 1)))

    w1p, w2p, b1p = [], [], []
    for u in range(5):
        w1p.append(np.ascontiguousarray(
            w1_all[u].astype(BF16).reshape(NHT, P, K1, P).transpose(0, 3, 2, 1)))
        w2p.append(np.ascontiguousarray(
            w2_all[u].astype(BF16).reshape(NOS, OSL, NHT, P).transpose(0, 3, 2, 1)))
        b1p.append(np.ascontiguousarray(b1_all[u].reshape(NHT, P).T))

    in_maps = []
    for c in range(8):
        e = c // 2
        in_maps.append({
            "xt": xts[c],
            "w1a": w1p[4], "w2a": w2p[4], "b1a": b1p[4],
            "w1b": w1p[e], "w2b": w2p[e], "b1b": b1p[e],
            "gates": np.ascontiguousarray(
                gate_val[c].reshape(TILES, P).T),
        })

    # host-side bias terms (b2/sb2 are added after the gate multiply)
    b2f = np.asarray(b2, np.float32)
    host_bias = (np.asarray(sb2, np.float32)[None]
                 + tw[:, 0:1] * b2f[ti[:, 0]]
                 + tw[:, 1:2] * b2f[ti[:, 1]])

    meta = (shared_slot, app_slot[0], app_slot[1], host_bias)
    return in_maps, meta


def _unpack(results, meta, Bsz, out_len):
    shared_slot, slot_a, slot_b, host_bias = meta
    y_flat = np.empty((8 * TILES * P, OUT_DIM), np.float32)
    for c in range(8):
        y_flat[c * SLOTS:(c + 1) * SLOTS] = (
            results[c]["y"].reshape(SLOTS, OUT_DIM).astype(np.float32))
    out = y_flat[shared_slot] + y_flat[slot_a] + y_flat[slot_b] + host_bias
    return out.reshape(Bsz, out_len, OUT_DIM)


def kernel(x, norm_w, router_w, w1, b1, w2, b2, sw1, sb1, sw2, sb2):
    x = np.asarray(x, dtype=np.float32)
    Bsz, S, D = x.shape          # [16, 1500, 1280]
    out_len = (S - 4) // 4 + 1   # 375

    in_maps, meta = _prepare(x, norm_w, router_w, w1, b1, w2, b2,
                             sw1, sb1, sw2, sb2)
    nc = _get_nc()
    # the axon-proxied execute occasionally hits a transient
    # NRT_EXEC_UNIT_UNRECOVERABLE; retry a few times
    last_exc = None
    for attempt in range(4):
        try:
            res = run_bass_kernel_spmd(nc, in_maps, core_ids=list(range(8)))
            break
        except Exception as exc:  # noqa: BLE001
            last_exc = exc
            import time
            time.sleep(5)
    else:
        raise last_exc

    return _unpack(res.results, meta, Bsz, out_len)


# revision 6
# speedup vs baseline: 1.0011x; 1.0011x over previous
"""MoE audio projector kernel for 8 Trainium2 NeuronCores — sparse top-2.

The reference computes all 4 experts densely and masks with top-2 gates.
Only the top-2 experts contribute, so the device only computes
  shared-MLP(token) + gate_a * expert_a-MLP(token) + gate_b * expert_b-MLP(token)
per token.  Host does routing (rmsnorm + router softmax + top-2, exactly
the reference formulas) and sorts tokens by expert; the device is a pure
bf16 matmul pipeline.

Sharding: 8 cores x 3 segments of 768 token-slots:
  seg0: 750 of the 6000 stacked tokens, shared expert (gate 1)
  seg1+seg2: ~1500 tokens routed to expert e = core//2 (each expert's
  ~3000 tokens split across 2 cores), gate = renormalized top-2 prob.
Per segment: mm1 [5120->2048] + gelu + mm2 [2048->2048], output scaled by
the per-token gate.  Host gathers the 3 partial outputs per token and sums.

All matmuls bf16 with fp32 PSUM accumulation (1 cycle/row on the PE).
"""
import sys

sys.path.insert(0, "/opt/trn_rl_repo")

import numpy as np
import ml_dtypes

import concourse.bass as bass
import concourse.mybir as mybir
import concourse.tile as tile
from concourse.bass_utils import run_bass_kernel_spmd

BF16 = np.dtype(ml_dtypes.bfloat16)

P = 128
IN_DIM = 5120
K1 = IN_DIM // P          # 40 contraction tiles for mm1
HID = 2048
NHT = HID // P            # 16 h tiles (mm1 out) == mm2 contraction tiles
OUT_DIM = 2048
OSL = 512                 # mm2 output slice (one PSUM bank)
NOS = OUT_DIM // OSL      # 4
E = 4
SEGS = 3
SEG_TILES = 6
SEG_TOK = SEG_TILES * P   # 768
TILES = SEGS * SEG_TILES  # 18
SLOTS = SEGS * SEG_TOK    # 2304 token slots per core
TC = 384                  # mm1 token chunk (2 per segment)
NTC = SEG_TOK // TC       # 2

N_TOK = 6000              # 16 batches x 375 stacked frames
SH_PER_CORE = N_TOK // 8  # 750 shared tokens per core
EXP_CAP_CORE = 2 * SEG_TOK  # 1536 expert tokens per core
EPS_NORM = 1e-6
EPS_GATE = 1e-6

MM_DT = mybir.dt.bfloat16
F32 = mybir.dt.float32


def split_excess_waits(nc, maxw=1):
    """This container's walrus build only accepts one sync-wait command on
    CTRL-class instructions (Drain) and two on regular ones; fan extra waits
    out onto preceding same-engine NoOps."""
    for f in nc.m.functions:
        for b in f.blocks:
            newlist = []
            for inst in b.instructions:
                lim = maxw
                si = inst.sync_info
                if si is not None and si.on_wait and len(si.on_wait) > lim:
                    waits = list(si.on_wait)
                    chunks = [waits[i:i + lim] for i in range(0, len(waits), lim)]
                    for ci, ch in enumerate(chunks[:-1]):
                        d = mybir.InstNoOp(
                            name=f"{inst.name}-waitsplit{ci}",
                            ins=[], outs=[],
                            sync_info=mybir.SyncInfo(on_wait=ch, on_update=[]),
                        )
                        d.engine = inst.engine
                        nc.register_instruction(d)
                        newlist.append(d)
                    si.on_wait = chunks[-1]
                newlist.append(inst)
            b.instructions = newlist


def build_nc():
    nc = bass.Bass()
    xt = nc.dram_tensor("xt", [SEGS, NTC, P, K1, TC], MM_DT, kind="ExternalInput")
    w1a = nc.dram_tensor("w1a", [NHT, P, K1, P], MM_DT, kind="ExternalInput")
    w1b = nc.dram_tensor("w1b", [NHT, P, K1, P], MM_DT, kind="ExternalInput")
    w2a = nc.dram_tensor("w2a", [NOS, P, NHT, OSL], MM_DT, kind="ExternalInput")
    w2b = nc.dram_tensor("w2b", [NOS, P, NHT, OSL], MM_DT, kind="ExternalInput")
    b1a = nc.dram_tensor("b1a", [P, NHT], F32, kind="ExternalInput")
    b1b = nc.dram_tensor("b1b", [P, NHT], F32, kind="ExternalInput")
    gates = nc.dram_tensor("gates", [P, TILES], F32, kind="ExternalInput")
    y = nc.dram_tensor("y", [TILES, P, OUT_DIM], MM_DT, kind="ExternalOutput")

    seg_w1 = [w1a, w1b, w1b]
    seg_w2 = [w2a, w2b, w2b]
    seg_b1 = [b1a, b1b, b1b]

    with tile.TileContext(nc) as tc:
        with (
            tc.tile_pool(name="singles", bufs=1) as singles,
            tc.tile_pool(name="xq", bufs=2) as xq_pool,
            tc.tile_pool(name="ht", bufs=1) as ht_pool,
            tc.tile_pool(name="w1s", bufs=2) as w1_pool,
            tc.tile_pool(name="w2s", bufs=3) as w2_pool,
            tc.tile_pool(name="b1s", bufs=2) as b1_pool,
            tc.tile_pool(name="outp", bufs=1) as out_pool,
            tc.tile_pool(name="psum1", bufs=2, space="PSUM") as psum1_pool,
            tc.tile_pool(name="psum2", bufs=3, space="PSUM") as psum2_pool,
        ):
            gates_sb = singles.tile([P, TILES], F32)
            nc.sync.dma_start(gates_sb, gates[:, :])

            for seg in range(SEGS):
                b1sb = b1_pool.tile([P, NHT], F32, tag="b1")
                nc.sync.dma_start(b1sb, seg_b1[seg][:, :])
                xTc = []
                for tcb in range(NTC):
                    xT = xq_pool.tile([P, K1, TC], MM_DT, tag="xt")
                    nc.sync.dma_start(xT, xt[seg, tcb])
                    xTc.append(xT)

                # ---- mm1 + gelu: hT[h, tok] for the whole segment ----
                hT = ht_pool.tile([P, NHT, SEG_TOK], MM_DT, tag="ht")
                for ht in range(NHT):
                    w1blk = w1_pool.tile([P, K1, P], MM_DT, tag="w1")
                    nc.sync.dma_start(w1blk, seg_w1[seg][ht])
                    for tcb in range(NTC):
                        ts = tcb * TC
                        ps1 = psum1_pool.tile([P, TC], F32, tag="ps1")
                        for ko in range(K1):
                            nc.tensor.matmul(
                                ps1,
                                lhsT=w1blk[:, ko, :],
                                rhs=xTc[tcb][:, ko, :],
                                start=(ko == 0), stop=(ko == K1 - 1),
                            )
                        nc.scalar.activation(
                            out=hT[:, ht, ts:ts + TC], in_=ps1,
                            func=mybir.ActivationFunctionType.Gelu,
                            bias=b1sb[:, ht:ht + 1], scale=1.0,
                        )

                # ---- mm2 + gate scale (out DMA per tile at the last slice) ----
                out_sb = out_pool.tile([P, SEG_TILES, OUT_DIM], MM_DT, tag="out")
                for os_ in range(NOS):
                    w2blk = w2_pool.tile([P, NHT, OSL], MM_DT, tag="w2")
                    nc.sync.dma_start(w2blk, seg_w2[seg][os_])
                    for tt in range(SEG_TILES):
                        ps2 = psum2_pool.tile([P, OSL], F32, tag="ps2")
                        for hk in range(NHT):
                            nc.tensor.matmul(
                                ps2,
                                lhsT=hT[:, hk, tt * P:(tt + 1) * P],
                                rhs=w2blk[:, hk, :],
                                start=(hk == 0), stop=(hk == NHT - 1),
                            )
                        col = seg * SEG_TILES + tt
                        nc.vector.tensor_scalar_mul(
                            out_sb[:, tt, os_ * OSL:(os_ + 1) * OSL],
                            ps2, gates_sb[:, col:col + 1],
                        )
                        if os_ == NOS - 1:
                            nc.sync.dma_start(
                                y[seg * SEG_TILES + tt], out_sb[:, tt, :])

    split_excess_waits(nc)
    return nc


_NC_CACHE = {}


def _get_nc():
    if "nc" not in _NC_CACHE:
        _NC_CACHE["nc"] = build_nc()
    return _NC_CACHE["nc"]


def _routing(x, norm_w, router_w):
    """Reference-exact rmsnorm + router softmax + top-2 (jax CPU, f32).

    Returns flat normalized tokens [N_TOK, IN_DIM] f32, top-2 indices
    [N_TOK, 2] and renormalized top-2 weights [N_TOK, 2].
    """
    try:
        import jax
        import jax.numpy as jnp

        cpu = jax.devices("cpu")[0]
        with jax.default_device(cpu):
            xj = jnp.asarray(x)
            B, S, D = xj.shape
            out_len = (S - 4) // 4 + 1
            xs = xj[:, :out_len * 4, :].reshape(B, out_len, D * 4)
            var = jnp.mean(xs * xs, axis=-1, keepdims=True)
            xs = jnp.asarray(norm_w) * (xs * jax.lax.rsqrt(var + EPS_NORM))
            flat = xs.reshape(-1, D * 4)
            logits = flat @ jnp.asarray(router_w).T
            probs = jax.nn.softmax(logits.astype(jnp.float32), axis=-1)
            tw, ti = jax.lax.top_k(probs, 2)
            tw = tw / (jnp.sum(tw, axis=-1, keepdims=True) + EPS_GATE)
            return (np.asarray(flat, np.float32), np.asarray(ti),
                    np.asarray(tw, np.float32))
    except Exception:
        pass
    # numpy fallback (same formulas, f32)
    xf = np.asarray(x, np.float32)
    B, S, D = xf.shape
    out_len = (S - 4) // 4 + 1
    xs = xf[:, :out_len * 4, :].reshape(B, out_len, D * 4)
    var = np.mean(xs * xs, axis=-1, keepdims=True, dtype=np.float32)
    xs = norm_w * (xs / np.sqrt(var + EPS_NORM))
    flat = np.ascontiguousarray(xs.reshape(-1, D * 4), dtype=np.float32)
    logits = flat @ np.asarray(router_w, np.float32).T
    lmax = logits.max(axis=-1, keepdims=True)
    ex = np.exp(logits - lmax)
    probs = ex / ex.sum(axis=-1, keepdims=True)
    ti = np.argsort(-probs, kind="stable", axis=-1)[:, :2]
    tw = np.take_along_axis(probs, ti, axis=-1)
    tw = tw / (tw.sum(axis=-1, keepdims=True) + EPS_GATE)
    return flat, ti.astype(np.int32), tw.astype(np.float32)


def _prepare(x, norm_w, router_w, w1, b1, w2, b2, sw1, sb1, sw2, sb2):
    """Host routing + packing.  Returns (in_maps, meta) where meta has the
    slot->token scatter info for _unpack."""
    flat, ti, tw = _routing(
        np.asarray(x, np.float32),
        np.asarray(norm_w, np.float32),
        np.asarray(router_w, np.float32),
    )

    w1_all = np.concatenate(
        [np.asarray(w1, np.float32), np.asarray(sw1, np.float32)[None]], axis=0)
    b1_all = np.concatenate(
        [np.asarray(b1, np.float32), np.asarray(sb1, np.float32)[None]], axis=0)
    w2_all = np.concatenate(
        [np.asarray(w2, np.float32), np.asarray(sw2, np.float32)[None]], axis=0)

    # ---- slot assignment ----
    # core c: seg0 = shared tokens [c*750, (c+1)*750); segs 1-2 = expert
    # c//2's tokens, first half of that expert's list for even c, second
    # half for odd c.
    tok_idx = np.full((8, SLOTS), N_TOK, np.int64)      # pad -> zero row
    gate_val = np.zeros((8, SLOTS), np.float32)

    sh = np.arange(N_TOK)
    shared_slot = (sh // SH_PER_CORE) * SLOTS + (sh % SH_PER_CORE)
    tok_idx.reshape(-1)[shared_slot] = sh
    gate_val.reshape(-1)[shared_slot] = 1.0

    app_slot = [np.empty(N_TOK, np.int64), np.empty(N_TOK, np.int64)]
    for e in range(E):
        m0 = ti[:, 0] == e
        m1 = ti[:, 1] == e
        L = np.flatnonzero(m0 | m1)
        if len(L) > 2 * EXP_CAP_CORE:
            raise ValueError(
                f"expert {e} has {len(L)} tokens > capacity {2 * EXP_CAP_CORE}")
        g = np.where(m0[L], tw[L, 0], tw[L, 1]).astype(np.float32)
        pos = np.arange(len(L))
        core = 2 * e + (pos >= EXP_CAP_CORE)
        within = pos - (pos >= EXP_CAP_CORE) * EXP_CAP_CORE
        slots = core * SLOTS + SEG_TOK + within
        tok_idx.reshape(-1)[slots] = L
        gate_val.reshape(-1)[slots] = g
        for j in (0, 1):
            sel = np.flatnonzero(ti[:, j] == e)
            ranks = np.searchsorted(L, sel)
            app_slot[j][sel] = slots[ranks]

    # ---- device arrays ----
    flat16 = np.zeros((N_TOK + 1, IN_DIM), BF16)
    flat16[:N_TOK] = flat.astype(BF16)

    # xt per core: [SEGS, NTC, P(kp), K1(ko), TC]
    xts = []
    for c in range(8):
        xg = flat16[tok_idx[c]]                              # [SLOTS, IN_DIM]
        xts.append(np.ascontiguousarray(
            xg.reshape(SEGS, NTC, TC, K1, P).transpose(0, 1, 4, 3, 2)))

    w1p, w2p, b1p = [], [], []
    for u in range(5):
        w1p.append(np.ascontiguousarray(
            w1_all[u].astype(BF16).reshape(NHT, P, K1, P).transpose(0, 3, 2, 1)))
        w2p.append(np.ascontiguousarray(
            w2_all[u].astype(BF16).reshape(NOS, OSL, NHT, P).transpose(0, 3, 2, 1)))
        b1p.append(np.ascontiguousarray(b1_all[u].reshape(NHT, P).T))

    in_maps = []
    for c in range(8):
        e = c // 2
        in_maps.append({
            "xt": xts[c],
            "w1a": w1p[4], "w2a": w2p[4], "b1a": b1p[4],
            "w1b": w1p[e], "w2b": w2p[e], "b1b": b1p[e],
            "gates": np.ascontiguousarray(
                gate_val[c].reshape(TILES, P).T),
        })

    # host-side bias terms (b2/sb2 are added after the gate multiply)
    b2f = np.asarray(b2, np.float32)
    host_bias = (np.asarray(sb2, np.float32)[None]
                 + tw[:, 0:1] * b2f[ti[:, 0]]
                 + tw[:, 1:2] * b2f[ti[:, 1]])

    meta = (shared_slot, app_slot[0], app_slot[1], host_bias)
    return in_maps, meta


def _unpack(results, meta, Bsz, out_len):
    shared_slot, slot_a, slot_b, host_bias = meta
    y_flat = np.empty((8 * TILES * P, OUT_DIM), np.float32)
    for c in range(8):
        y_flat[c * SLOTS:(c + 1) * SLOTS] = (
            results[c]["y"].reshape(SLOTS, OUT_DIM).astype(np.float32))
    out = y_flat[shared_slot] + y_flat[slot_a] + y_flat[slot_b] + host_bias
    return out.reshape(Bsz, out_len, OUT_DIM)


def kernel(x, norm_w, router_w, w1, b1, w2, b2, sw1, sb1, sw2, sb2):
    x = np.asarray(x, dtype=np.float32)
    Bsz, S, D = x.shape          # [16, 1500, 1280]
    out_len = (S - 4) // 4 + 1   # 375

    in_maps, meta = _prepare(x, norm_w, router_w, w1, b1, w2, b2,
                             sw1, sb1, sw2, sb2)
    nc = _get_nc()
    # the axon-proxied execute occasionally hits a transient
    # NRT_EXEC_UNIT_UNRECOVERABLE; retry a few times
    last_exc = None
    for attempt in range(4):
        try:
            res = run_bass_kernel_spmd(nc, in_maps, core_ids=list(range(8)))
            break
        except Exception as exc:  # noqa: BLE001
            last_exc = exc
            import time
            time.sleep(5)
    else:
        raise last_exc

    return _unpack(res.results, meta, Bsz, out_len)
